# revision 36
# baseline (speedup 1.0000x reference)
"""Trainium2 Bass kernel for nn_Block_41077067219413.

Reference computation (B=2048, D=dim_in=4096, J=dim_out=4096):
    xf = x.astype(f32)                 # (B, D) in {0,1}
    mf = masks.astype(f32)             # (D, J) in {0,1}
    sums = xf @ mf + (1-xf) @ (1-mf)   # XNOR popcount over D
    out  = sums > thresholds[None, :]  # (B, J) bool

Identity used: with x' = 2x-1 in {-1,+1} and m in {0,1},
    A[b,j] = sum_k x'[b,k] * m[k,j]
    sums   = A + D - rowsum_x[b]
    out    = A - th[j] > rowsum_x[b] - D

One fp8 GEMM per core (batch-sharded 8 ways).  masks bytes {0,1} are DMA'd
raw and bitcast to fp8e4, where 0x01 is the subnormal eps=2^-9 -- the GEMM
computes eps*A exactly (integers scaled by eps are exact in fp32).
Thresholds are folded into the GEMM as 4 extra contraction rows carrying
base-8 digits of th with eps-scaled weights, so PSUM = eps*(A - th).
Epilogue: single per-partition-scalar is_gt vs eps*(rowsum_x - D) -> uint8.
"""

import numpy as np

B, D, J = 2048, 4096, 4096
NCORES = 8
BL = B // NCORES          # 256 rows per core
P = 128
KT = D // P               # 32 k-tiles
NB = BL // P              # 2 b-tiles per core
JN = 512                  # matmul free-dim tile (one PSUM bank)
JT = J // JN              # 8 j-tiles
THC = 1024                # threshold digit-build chunk width

_cache = {}


def _build():
    import concourse.bacc as bacc
    import concourse.mybir as mybir
    import concourse.tile as tile

    dt = mybir.dt
    f8 = dt.float8e4
    f32 = dt.float32
    AF = mybir.ActivationFunctionType
    ALU = mybir.AluOpType
    DR = mybir.MatmulPerfMode.DoubleRow

    nc = bacc.Bacc("TRN2", target_bir_lowering=False, debug=False,
                   num_devices=NCORES)

    x_d = nc.dram_tensor("x", [BL, D], dt.int32, kind="ExternalInput")
    m_d = nc.dram_tensor("masks", [D, J], dt.uint8, kind="ExternalInput")
    th_d = nc.dram_tensor("th", [1, J], dt.int32, kind="ExternalInput")
    cst8_d = nc.dram_tensor("cst8", [4, P], f8, kind="ExternalInput")
    ident_d = nc.dram_tensor("ident", [P, P], f8, kind="ExternalInput")
    csts_d = nc.dram_tensor("csts", [4, 2], dt.int32, kind="ExternalInput")
    o_d = nc.dram_tensor("out", [BL, J], dt.uint8, kind="ExternalOutput")

    with tile.TileContext(nc) as tc:
        with (
            tc.tile_pool(name="const", bufs=1) as constp,
            tc.tile_pool(name="mask", bufs=1) as maskp,
            tc.tile_pool(name="xt", bufs=1) as xtp,
            tc.tile_pool(name="ob", bufs=2) as obufp,
            tc.tile_pool(name="xio", bufs=4) as xiop,
            tc.tile_pool(name="thp", bufs=1) as thp,
            tc.tile_pool(name="xpm", bufs=1) as xpmp,
        ):
            # ---- x loads: 8 quarter-tiles, all on the SWDGE ring ahead of
            # masks -- deep in-flight pipelining hits full DMA rate
            NQ = 4
            HD = D // NQ
            xio = {}
            for b in range(NB):
                for h in range(NQ):
                    xi = xiop.tile([P, HD], dt.int32, tag="xi",
                                   name=f"xi{b}_{h}")
                    nc.gpsimd.dma_start(
                        xi[:], x_d[b * P:(b + 1) * P, h * HD:(h + 1) * HD])
                    xio[(b, h)] = xi

            # ---- const tables (small sync DMAs)
            wstar = constp.tile([4, P], f8)
            nc.scalar.dma_start(wstar[:], cst8_d[:])
            identity8 = constp.tile([P, P], f8)
            nc.scalar.dma_start(identity8[:], ident_d[:])
            shiftands = constp.tile([4, 2], dt.int32)
            nc.scalar.dma_start(shiftands[:], csts_d[:])

            neg1 = constp.tile([P, 1], f32)
            nc.vector.memset(neg1[:], -1.0)
            actwarm = constp.tile([P, 1], f32)
            nc.scalar.activation(actwarm[:], neg1[:], AF.Identity,
                                 bias=neg1[:], scale=1.0)
            rxe = constp.tile([P, NB], f32)
            dig8 = constp.tile([4, J], f8)

            # ---- masks: raw uint8 DMA, k-pair layout (bitcast fp8 at use)
            xgate = constp.tile([1, 4], dt.int32)
            nc.gpsimd.tensor_copy(xgate[:], xio[(1, NQ - 1)][0:1, 0:4])
            JH = J // 2
            mask_tiles = {}
            for jh in range(2):
                for kp in range(KT // 2):
                    mt = maskp.tile([P, 2, JH], dt.uint8,
                                    name=f"mk{jh}_{kp}", tag=f"mk{jh}_{kp}")
                    src = m_d[kp * 2 * P:(kp + 1) * 2 * P,
                              jh * JH:(jh + 1) * JH].rearrange(
                        "(ko ki) j -> ki ko j", ki=P)
                    nc.gpsimd.dma_start(mt[:], src)
                    mask_tiles[(jh, kp)] = mt

            # ---- thresholds -> base-8 digit rows [4, J] fp8 (chunked temps)
            if True:
                for c0 in range(0, J, THC):
                    th4 = thp.tile([4, THC], dt.int32, tag="th4",
                                   name=f"th4_{c0}")
                    for i in range(4):
                        nc.scalar.dma_start(th4[i:i + 1, :],
                                            th_d[:, c0:c0 + THC])
                    dig_i = thp.tile([4, THC], dt.int32, tag="dig_i",
                                     name=f"dig_i_{c0}")
                    nc.vector.tensor_scalar(
                        dig_i[:], th4[:], shiftands[:, 0:1],
                        shiftands[:, 1:2],
                        op0=ALU.arith_shift_right, op1=ALU.bitwise_and)
                    nc.vector.tensor_copy(dig8[:, c0:c0 + THC], dig_i[:])

            # ---- x: int32 -> fp8 {-1,+1} + rowsum; PE-transpose into
            # xT [128, KT, 256] (dim1 = k-tile, dim2 = b columns)
            xT = xtp.tile([P, KT, NB * P], f8)
            with (
                tc.tile_pool(name="pstp", bufs=4, space="PSUM") as pstp,
            ):
                KQ = KT // NQ
                for b in range(NB):
                    rxas = []
                    for h in range(NQ):
                        xpm = xpmp.tile([P, HD], f8, tag=f"xpm{b}_{h}",
                                        name=f"xpm{b}_{h}")
                        rxa = xpmp.tile([P, 1], f32, tag=f"rxa{b}_{h}",
                                        name=f"rxa{b}_{h}")
                        nc.scalar.activation(
                            xpm[:], xio[(b, h)][:], AF.Identity,
                            bias=neg1[:], scale=2.0, accum_out=rxa[:])
                        rxas.append(rxa)
                        for pp in range(KQ // 2):
                            pst = pstp.tile([P, 2, P, 2], f8, tag="pst")
                            for q in range(2):
                                k = 2 * pp + q
                                nc.tensor.transpose(
                                    pst[:, q, :, 0],
                                    xpm[:, k * P:(k + 1) * P],
                                    identity8[:])
                            kk = h * KQ + 2 * pp
                            nc.vector.tensor_copy(
                                xT[:, kk:kk + 2, b * P:(b + 1) * P],
                                pst[:, :, :, 0])
                    # rxe_b = eps*(rowsum_x - D) = (sum accums)/1024 - 4
                    nc.vector.tensor_tensor(
                        rxas[0][:], rxas[0][:], rxas[1][:], op=ALU.add)
                    nc.vector.tensor_tensor(
                        rxas[2][:], rxas[2][:], rxas[3][:], op=ALU.add)
                    nc.vector.tensor_tensor(
                        rxas[0][:], rxas[0][:], rxas[2][:], op=ALU.add)
                    nc.vector.tensor_scalar(
                        rxe[:, b:b + 1], rxas[0][:], 1.0 / 1024.0, -4.0,
                        op0=ALU.mult, op1=ALU.add)

            # ---- main GEMM + fused threshold + epilogue
            obs = [obufp.tile([P, J], dt.uint8, tag=f"ob{b}", name=f"ob{b}")
                   for b in range(NB)]
            with tc.tile_pool(name="psacc", bufs=1, space="PSUM") as psacc:
                KH = KT // 4     # 8: first half of kp range (kp-major)
                for jh in range(2):
                    ps = {}
                    for b in range(NB):
                        for j4 in range(4):
                            ps[(b, j4)] = psacc.tile(
                                [P, JN], f32, name=f"acc{jh}_{b}_{j4}",
                                tag=f"acc{b}_{j4}")
                    for kp in range(KH):
                        mt = mask_tiles[(jh, kp)]
                        for b in range(NB):
                            w = xT[:, 2 * kp:2 * kp + 2, b * P:(b + 1) * P]
                            for j4 in range(4):
                                nc.tensor.matmul(
                                    ps[(b, j4)][:], w,
                                    mt[:, :,
                                       j4 * JN:(j4 + 1) * JN].bitcast(f8),
                                    start=(kp == 0), stop=False,
                                    perf_mode=DR)
                        if kp == 2:
                            # fold thresholds: psum -= eps*th
                            for b in range(NB):
                                for j4 in range(4):
                                    jj = jh * (J // 2) + j4 * JN
                                    nc.tensor.matmul(
                                        ps[(b, j4)][:], wstar[:],
                                        dig8[:, jj:jj + JN],
                                        start=False, stop=False,
                                        skip_group_check=True)
                    # second k-half group-major: groups retire staggered so
                    # the is_gt epilogue overlaps remaining matmuls
                    for b in range(NB):
                        for j4 in range(4):
                            jj = jh * (J // 2) + j4 * JN
                            w = None
                            for kp in range(KH, KT // 2):
                                nc.tensor.matmul(
                                    ps[(b, j4)][:],
                                    xT[:, 2 * kp:2 * kp + 2,
                                       b * P:(b + 1) * P],
                                    mask_tiles[(jh, kp)][
                                        :, :,
                                        j4 * JN:(j4 + 1) * JN].bitcast(f8),
                                    start=False, stop=(kp == KT // 2 - 1),
                                    perf_mode=DR)
                            nc.vector.tensor_scalar(
                                obs[b][:, jj:jj + JN], ps[(b, j4)][:],
                                rxe[:, b:b + 1], None, op0=ALU.is_gt)
                            nc.sync.dma_start(
                                o_d[b * P:(b + 1) * P, jj:jj + JN],
                                obs[b][:, jj:jj + JN])

    nc.compile()
    return nc


def _get_nc():
    if "nc" not in _cache:
        _cache["nc"] = _build()
    return _cache["nc"]


def _cst8():
    import ml_dtypes
    # eps-scaled digit weights: -eps*8^i per digit row (row 3 holds 8*d3,
    # so its weight is -eps*512/8 = -2^-3)
    w = np.array([-2.0 ** -9, -2.0 ** -6, -2.0 ** -3, -2.0 ** -3],
                 dtype=np.float32)
    return np.broadcast_to(w[:, None], (4, P)).astype(ml_dtypes.float8_e4m3)


def _ident():
    import ml_dtypes
    return np.eye(P, dtype=np.float32).astype(ml_dtypes.float8_e4m3)


def _csts():
    return np.array([[0, 7], [3, 7], [6, 7], [6, 56]], dtype=np.int32)


def run(x, masks, thresholds, trace=False):
    """Run the SPMD kernel on 8 cores. Returns (out_bool, BassKernelResults)."""
    from concourse.bass_utils import run_bass_kernel_spmd

    nc = _get_nc()
    m_u8 = np.ascontiguousarray(masks.view(np.uint8))
    th = np.ascontiguousarray(thresholds.reshape(1, J).astype(np.int32))
    in_maps = []
    for c in range(NCORES):
        in_maps.append({
            "x": np.ascontiguousarray(x[c * BL:(c + 1) * BL, :]),
            "masks": m_u8,
            "th": th,
            "cst8": _cst8(),
            "ident": _ident(),
            "csts": _csts(),
        })
    res = run_bass_kernel_spmd(nc, in_maps, core_ids=list(range(NCORES)),
                               trace=trace)
    out = np.concatenate([r["out"] for r in res.results], axis=0)
    return out.view(np.bool_), res


def kernel(x, masks, thresholds):
    x = np.asarray(x)
    masks = np.asarray(masks)
    thresholds = np.asarray(thresholds)
    out, _ = run(x, masks, thresholds, trace=False)
    return out


# revision 37
# speedup vs baseline: 1.0485x; 1.0485x over previous
"""Trainium2 Bass kernel for nn_Block_41077067219413.

Reference computation (B=2048, D=dim_in=4096, J=dim_out=4096):
    xf = x.astype(f32)                 # (B, D) in {0,1}
    mf = masks.astype(f32)             # (D, J) in {0,1}
    sums = xf @ mf + (1-xf) @ (1-mf)   # XNOR popcount over D
    out  = sums > thresholds[None, :]  # (B, J) bool

Identity used: with x' = 2x-1 in {-1,+1} and m in {0,1},
    A[b,j] = sum_k x'[b,k] * m[k,j]
    sums   = A + D - rowsum_x[b]
    out    = A - th[j] > rowsum_x[b] - D

One fp8 GEMM per core (batch-sharded 8 ways).  masks bytes {0,1} are DMA'd
raw and bitcast to fp8e4, where 0x01 is the subnormal eps=2^-9 -- the GEMM
computes eps*A exactly (integers scaled by eps are exact in fp32).
Thresholds are folded into the GEMM as 4 extra contraction rows carrying
base-8 digits of th with eps-scaled weights, so PSUM = eps*(A - th).
Epilogue: single per-partition-scalar is_gt vs eps*(rowsum_x - D) -> uint8.
"""

import numpy as np

B, D, J = 2048, 4096, 4096
NCORES = 8
BL = B // NCORES          # 256 rows per core
P = 128
KT = D // P               # 32 k-tiles
NB = BL // P              # 2 b-tiles per core
JN = 512                  # matmul free-dim tile (one PSUM bank)
JT = J // JN              # 8 j-tiles
THC = 1024                # threshold digit-build chunk width

_cache = {}


def _build():
    import concourse.bacc as bacc
    import concourse.mybir as mybir
    import concourse.tile as tile

    dt = mybir.dt
    f8 = dt.float8e4
    f32 = dt.float32
    AF = mybir.ActivationFunctionType
    ALU = mybir.AluOpType
    DR = mybir.MatmulPerfMode.DoubleRow

    nc = bacc.Bacc("TRN2", target_bir_lowering=False, debug=False,
                   num_devices=NCORES)

    x_d = nc.dram_tensor("x", [BL, D], dt.int32, kind="ExternalInput")
    m_d = nc.dram_tensor("masks", [D, J], dt.uint8, kind="ExternalInput")
    th_d = nc.dram_tensor("th", [1, J], dt.int32, kind="ExternalInput")
    cst8_d = nc.dram_tensor("cst8", [4, P], f8, kind="ExternalInput")
    ident_d = nc.dram_tensor("ident", [P, P], f8, kind="ExternalInput")
    csts_d = nc.dram_tensor("csts", [4, 2], dt.int32, kind="ExternalInput")
    o_d = nc.dram_tensor("out", [BL, J], dt.uint8, kind="ExternalOutput")

    with tile.TileContext(nc) as tc:
        with (
            tc.tile_pool(name="const", bufs=1) as constp,
            tc.tile_pool(name="mask", bufs=1) as maskp,
            tc.tile_pool(name="xt", bufs=1) as xtp,
            tc.tile_pool(name="ob", bufs=2) as obufp,
            tc.tile_pool(name="xio", bufs=1) as xiop,
            tc.tile_pool(name="thp", bufs=1) as thp,
            tc.tile_pool(name="xpm", bufs=1) as xpmp,
        ):
            # ---- x loads: 8 quarter-tiles, all on the SWDGE ring ahead of
            # masks -- deep in-flight pipelining hits full DMA rate
            NQ = 4
            HD = D // NQ
            xio = {}
            for b in range(NB):
                for h in range(NQ):
                    xi = xiop.tile([P, HD], dt.int32, tag=f"xi{b}_{h}",
                                   name=f"xi{b}_{h}")
                    nc.gpsimd.dma_start(
                        xi[:], x_d[b * P:(b + 1) * P, h * HD:(h + 1) * HD])
                    xio[(b, h)] = xi

            # ---- const tables (small sync DMAs)
            wstar = constp.tile([4, P], f8)
            nc.scalar.dma_start(wstar[:], cst8_d[:])
            identity8 = constp.tile([P, P], f8)
            nc.scalar.dma_start(identity8[:], ident_d[:])
            shiftands = constp.tile([4, 2], dt.int32)
            nc.scalar.dma_start(shiftands[:], csts_d[:])

            neg1 = constp.tile([P, 1], f32)
            nc.vector.memset(neg1[:], -1.0)
            actwarm = constp.tile([P, 1], f32)
            nc.scalar.activation(actwarm[:], neg1[:], AF.Identity,
                                 bias=neg1[:], scale=1.0)
            rxe = constp.tile([P, NB], f32)
            dig8 = constp.tile([4, J], f8)

            # ---- masks: raw uint8 DMA, k-pair layout (bitcast fp8 at use)
            xgate = constp.tile([1, 4], dt.int32)
            nc.gpsimd.tensor_copy(xgate[:], xio[(1, NQ - 1)][0:1, 0:4])
            JH = J // 2
            mask_tiles = {}
            for jh in range(2):
                for kp in range(KT // 2):
                    mt = maskp.tile([P, 2, JH], dt.uint8,
                                    name=f"mk{jh}_{kp}", tag=f"mk{jh}_{kp}")
                    src = m_d[kp * 2 * P:(kp + 1) * 2 * P,
                              jh * JH:(jh + 1) * JH].rearrange(
                        "(ko ki) j -> ki ko j", ki=P)
                    nc.gpsimd.dma_start(mt[:], src)
                    mask_tiles[(jh, kp)] = mt

            # ---- thresholds -> base-8 digit rows [4, J] fp8 (chunked temps)
            if True:
                for c0 in range(0, J, THC):
                    th4 = thp.tile([4, THC], dt.int32, tag="th4",
                                   name=f"th4_{c0}")
                    for i in range(4):
                        nc.scalar.dma_start(th4[i:i + 1, :],
                                            th_d[:, c0:c0 + THC])
                    dig_i = thp.tile([4, THC], dt.int32, tag="dig_i",
                                     name=f"dig_i_{c0}")
                    nc.vector.tensor_scalar(
                        dig_i[:], th4[:], shiftands[:, 0:1],
                        shiftands[:, 1:2],
                        op0=ALU.arith_shift_right, op1=ALU.bitwise_and)
                    nc.vector.tensor_copy(dig8[:, c0:c0 + THC], dig_i[:])

            # ---- x: int32 -> fp8 {-1,+1} + rowsum; PE-transpose into
            # xT [128, KT, 256] (dim1 = k-tile, dim2 = b columns)
            xT = xtp.tile([P, KT, NB * P], f8)
            with (
                tc.tile_pool(name="pstp", bufs=4, space="PSUM") as pstp,
            ):
                KQ = KT // NQ
                for b in range(NB):
                    rxas = []
                    for h in range(NQ):
                        xpm = xpmp.tile([P, HD], f8, tag=f"xpm{b}_{h}",
                                        name=f"xpm{b}_{h}")
                        rxa = xpmp.tile([P, 1], f32, tag=f"rxa{b}_{h}",
                                        name=f"rxa{b}_{h}")
                        nc.scalar.activation(
                            xpm[:], xio[(b, h)][:], AF.Identity,
                            bias=neg1[:], scale=2.0, accum_out=rxa[:])
                        rxas.append(rxa)
                        for pp in range(KQ // 2):
                            pst = pstp.tile([P, 2, P, 2], f8, tag="pst")
                            for q in range(2):
                                k = 2 * pp + q
                                nc.tensor.transpose(
                                    pst[:, q, :, 0],
                                    xpm[:, k * P:(k + 1) * P],
                                    identity8[:])
                            kk = h * KQ + 2 * pp
                            nc.vector.tensor_copy(
                                xT[:, kk:kk + 2, b * P:(b + 1) * P],
                                pst[:, :, :, 0])
                    # rxe_b = eps*(rowsum_x - D) = (sum accums)/1024 - 4
                    nc.vector.tensor_tensor(
                        rxas[0][:], rxas[0][:], rxas[1][:], op=ALU.add)
                    nc.vector.tensor_tensor(
                        rxas[2][:], rxas[2][:], rxas[3][:], op=ALU.add)
                    nc.vector.tensor_tensor(
                        rxas[0][:], rxas[0][:], rxas[2][:], op=ALU.add)
                    nc.vector.tensor_scalar(
                        rxe[:, b:b + 1], rxas[0][:], 1.0 / 1024.0, -4.0,
                        op0=ALU.mult, op1=ALU.add)

            # ---- main GEMM + fused threshold + epilogue
            obs = [obufp.tile([P, J], dt.uint8, tag=f"ob{b}", name=f"ob{b}")
                   for b in range(NB)]
            with tc.tile_pool(name="psacc", bufs=1, space="PSUM") as psacc:
                KH = KT // 4     # 8: first half of kp range (kp-major)
                for jh in range(2):
                    ps = {}
                    for b in range(NB):
                        for j4 in range(4):
                            ps[(b, j4)] = psacc.tile(
                                [P, JN], f32, name=f"acc{jh}_{b}_{j4}",
                                tag=f"acc{b}_{j4}")
                    for kp in range(KH):
                        mt = mask_tiles[(jh, kp)]
                        for b in range(NB):
                            w = xT[:, 2 * kp:2 * kp + 2, b * P:(b + 1) * P]
                            for j4 in range(4):
                                nc.tensor.matmul(
                                    ps[(b, j4)][:], w,
                                    mt[:, :,
                                       j4 * JN:(j4 + 1) * JN].bitcast(f8),
                                    start=(kp == 0), stop=False,
                                    perf_mode=DR)
                        if kp == 2:
                            # fold thresholds: psum -= eps*th
                            for b in range(NB):
                                for j4 in range(4):
                                    jj = jh * (J // 2) + j4 * JN
                                    nc.tensor.matmul(
                                        ps[(b, j4)][:], wstar[:],
                                        dig8[:, jj:jj + JN],
                                        start=False, stop=False,
                                        skip_group_check=True)
                    # second k-half group-major: groups retire staggered so
                    # the is_gt epilogue overlaps remaining matmuls
                    for b in range(NB):
                        for j4 in range(4):
                            jj = jh * (J // 2) + j4 * JN
                            w = None
                            for kp in range(KH, KT // 2):
                                nc.tensor.matmul(
                                    ps[(b, j4)][:],
                                    xT[:, 2 * kp:2 * kp + 2,
                                       b * P:(b + 1) * P],
                                    mask_tiles[(jh, kp)][
                                        :, :,
                                        j4 * JN:(j4 + 1) * JN].bitcast(f8),
                                    start=False, stop=(kp == KT // 2 - 1),
                                    perf_mode=DR)
                            nc.vector.tensor_scalar(
                                obs[b][:, jj:jj + JN], ps[(b, j4)][:],
                                rxe[:, b:b + 1], None, op0=ALU.is_gt)
                            nc.sync.dma_start(
                                o_d[b * P:(b + 1) * P, jj:jj + JN],
                                obs[b][:, jj:jj + JN])

    nc.compile()
    return nc


def _get_nc():
    if "nc" not in _cache:
        _cache["nc"] = _build()
    return _cache["nc"]


def _cst8():
    import ml_dtypes
    # eps-scaled digit weights: -eps*8^i per digit row (row 3 holds 8*d3,
    # so its weight is -eps*512/8 = -2^-3)
    w = np.array([-2.0 ** -9, -2.0 ** -6, -2.0 ** -3, -2.0 ** -3],
                 dtype=np.float32)
    return np.broadcast_to(w[:, None], (4, P)).astype(ml_dtypes.float8_e4m3)


def _ident():
    import ml_dtypes
    return np.eye(P, dtype=np.float32).astype(ml_dtypes.float8_e4m3)


def _csts():
    return np.array([[0, 7], [3, 7], [6, 7], [6, 56]], dtype=np.int32)


def run(x, masks, thresholds, trace=False):
    """Run the SPMD kernel on 8 cores. Returns (out_bool, BassKernelResults)."""
    from concourse.bass_utils import run_bass_kernel_spmd

    nc = _get_nc()
    m_u8 = np.ascontiguousarray(masks.view(np.uint8))
    th = np.ascontiguousarray(thresholds.reshape(1, J).astype(np.int32))
    in_maps = []
    for c in range(NCORES):
        in_maps.append({
            "x": np.ascontiguousarray(x[c * BL:(c + 1) * BL, :]),
            "masks": m_u8,
            "th": th,
            "cst8": _cst8(),
            "ident": _ident(),
            "csts": _csts(),
        })
    res = run_bass_kernel_spmd(nc, in_maps, core_ids=list(range(NCORES)),
                               trace=trace)
    out = np.concatenate([r["out"] for r in res.results], axis=0)
    return out.view(np.bool_), res


def kernel(x, masks, thresholds):
    x = np.asarray(x)
    masks = np.asarray(masks)
    thresholds = np.asarray(thresholds)
    out, _ = run(x, masks, thresholds, trace=False)
    return out


# revision 39
# speedup vs baseline: 1.1636x; 1.1097x over previous
"""Trainium2 Bass kernel for nn_Block_41077067219413.

Reference computation (B=2048, D=dim_in=4096, J=dim_out=4096):
    xf = x.astype(f32)                 # (B, D) in {0,1}
    mf = masks.astype(f32)             # (D, J) in {0,1}
    sums = xf @ mf + (1-xf) @ (1-mf)   # XNOR popcount over D
    out  = sums > thresholds[None, :]  # (B, J) bool

Identity used: with x' = 2x-1 in {-1,+1} and m in {0,1},
    A[b,j] = sum_k x'[b,k] * m[k,j]
    sums   = A + D - rowsum_x[b]
    out    = A - th[j] > rowsum_x[b] - D

One fp8 GEMM per core (batch-sharded 8 ways).  masks bytes {0,1} are DMA'd
raw and bitcast to fp8e4, where 0x01 is the subnormal eps=2^-9 -- the GEMM
computes eps*A exactly (integers scaled by eps are exact in fp32).
Thresholds are folded into the GEMM as 4 extra contraction rows carrying
base-8 digits of th with eps-scaled weights, so PSUM = eps*(A - th).
Epilogue: single per-partition-scalar is_gt vs eps*(rowsum_x - D) -> uint8.
"""

import numpy as np

B, D, J = 2048, 4096, 4096
NCORES = 8
BL = B // NCORES          # 256 rows per core
P = 128
KT = D // P               # 32 k-tiles
NB = BL // P              # 2 b-tiles per core
JN = 512                  # matmul free-dim tile (one PSUM bank)
JT = J // JN              # 8 j-tiles
THC = 1024                # threshold digit-build chunk width

_cache = {}


def _build():
    import concourse.bacc as bacc
    import concourse.mybir as mybir
    import concourse.tile as tile

    dt = mybir.dt
    f8 = dt.float8e4
    f32 = dt.float32
    AF = mybir.ActivationFunctionType
    ALU = mybir.AluOpType
    DR = mybir.MatmulPerfMode.DoubleRow

    nc = bacc.Bacc("TRN2", target_bir_lowering=False, debug=False,
                   num_devices=NCORES)

    x_d = nc.dram_tensor("x", [BL, D], dt.int32, kind="ExternalInput")
    m_d = nc.dram_tensor("masks", [D, J], dt.uint8, kind="ExternalInput")
    th_d = nc.dram_tensor("th", [1, J], dt.int32, kind="ExternalInput")
    cst8_d = nc.dram_tensor("cst8", [4, P], f8, kind="ExternalInput")
    ident_d = nc.dram_tensor("ident", [P, P], f8, kind="ExternalInput")
    csts_d = nc.dram_tensor("csts", [4, 2], dt.int32, kind="ExternalInput")
    o_d = nc.dram_tensor("out", [BL, J], dt.uint8, kind="ExternalOutput")

    with tile.TileContext(nc) as tc:
        with (
            tc.tile_pool(name="const", bufs=1) as constp,
            tc.tile_pool(name="mask", bufs=1) as maskp,
            tc.tile_pool(name="xt", bufs=1) as xtp,
            tc.tile_pool(name="ob", bufs=2) as obufp,
            tc.tile_pool(name="xio", bufs=4) as xiop,
            tc.tile_pool(name="thp", bufs=1) as thp,
            tc.tile_pool(name="xpm", bufs=1) as xpmp,
        ):
            # ---- x loads lead the sync (HWDGE) queue: critical path to PE
            # (half-tiles so the convert/transpose chain starts early)
            HD = D // 2
            xio = {}
            for b in range(NB):
                for h in range(2):
                    xi = xiop.tile([P, HD], dt.int32, tag="xi",
                                   name=f"xi{b}_{h}")
                    nc.gpsimd.dma_start(
                        xi[:], x_d[b * P:(b + 1) * P, h * HD:(h + 1) * HD])
                    xio[(b, h)] = xi

            # ---- const tables (small sync DMAs)
            wstar = constp.tile([4, P], f8)
            nc.scalar.dma_start(wstar[:], cst8_d[:])
            identity8 = constp.tile([P, P], f8)
            nc.scalar.dma_start(identity8[:], ident_d[:])
            shiftands = constp.tile([4, 2], dt.int32)
            nc.scalar.dma_start(shiftands[:], csts_d[:])

            neg1 = constp.tile([P, 1], f32)
            nc.vector.memset(neg1[:], -1.0)
            actwarm = constp.tile([P, 1], f32)
            nc.scalar.activation(actwarm[:], neg1[:], AF.Identity,
                                 bias=neg1[:], scale=1.0)
            rxe = constp.tile([P, NB], f32)
            dig8 = constp.tile([4, J], f8)

            # ---- masks: raw uint8 DMA, k-pair layout (bitcast fp8 at use)
            xgate = constp.tile([1, 4], dt.int32)
            nc.gpsimd.tensor_copy(xgate[:], xio[(1, 1)][0:1, 0:4])
            JH = J // 2
            mask_tiles = {}
            for jh in range(2):
                for kp in range(KT // 2):
                    mt = maskp.tile([P, 2, JH], dt.uint8,
                                    name=f"mk{jh}_{kp}", tag=f"mk{jh}_{kp}")
                    src = m_d[kp * 2 * P:(kp + 1) * 2 * P,
                              jh * JH:(jh + 1) * JH].rearrange(
                        "(ko ki) j -> ki ko j", ki=P)
                    nc.gpsimd.dma_start(mt[:], src)
                    mask_tiles[(jh, kp)] = mt

            # ---- thresholds -> base-8 digit rows [4, J] fp8 (chunked temps)
            if True:
                for c0 in range(0, J, THC):
                    th4 = thp.tile([4, THC], dt.int32, tag="th4",
                                   name=f"th4_{c0}")
                    for i in range(4):
                        nc.scalar.dma_start(th4[i:i + 1, :],
                                            th_d[:, c0:c0 + THC])
                    dig_i = thp.tile([4, THC], dt.int32, tag="dig_i",
                                     name=f"dig_i_{c0}")
                    nc.vector.tensor_scalar(
                        dig_i[:], th4[:], shiftands[:, 0:1],
                        shiftands[:, 1:2],
                        op0=ALU.arith_shift_right, op1=ALU.bitwise_and)
                    nc.vector.tensor_copy(dig8[:, c0:c0 + THC], dig_i[:])

            # ---- x: int32 -> fp8 {-1,+1} + rowsum (convert now,
            # transposes emitted inside the main section where they share
            # PSUM slots with the b1 accumulator tags)
            xT = xtp.tile([P, KT, NB * P], f8)
            xpms = {}
            for b in range(NB):
                rxas = []
                for h in range(2):
                    xpm = xpmp.tile([P, HD], f8, tag=f"xpm{b}_{h}",
                                    name=f"xpm{b}_{h}")
                    rxa = xpmp.tile([P, 1], f32, tag=f"rxa{b}_{h}",
                                    name=f"rxa{b}_{h}")
                    nc.scalar.activation(
                        xpm[:], xio[(b, h)][:], AF.Identity,
                        bias=neg1[:], scale=2.0, accum_out=rxa[:])
                    xpms[(b, h)] = xpm
                    rxas.append(rxa)
                nc.vector.tensor_tensor(
                    rxas[0][:], rxas[0][:], rxas[1][:], op=ALU.add)
                nc.vector.tensor_scalar(
                    rxe[:, b:b + 1], rxas[0][:], 1.0 / 1024.0, -4.0,
                    op0=ALU.mult, op1=ALU.add)

            # ---- main GEMM + fused threshold + epilogue
            obs = [obufp.tile([P, J], dt.uint8, tag=f"ob{b}", name=f"ob{b}")
                   for b in range(NB)]
            with tc.tile_pool(name="psacc", bufs=1, space="PSUM") as psacc:

                def transposes(b):
                    # pst tiles share the b1-accumulator slots (acc1_*)
                    for h in range(2):
                        for pp in range(KT // 4):
                            pst = psacc.tile(
                                [P, 2, P, 2], f8,
                                tag=f"acc1_{pp % 4}",
                                name=f"pst{b}_{h}_{pp}")
                            for q in range(2):
                                k = 2 * pp + q
                                nc.tensor.transpose(
                                    pst[:, q, :, 0],
                                    xpms[(b, h)][:, k * P:(k + 1) * P],
                                    identity8[:])
                            kk = h * (KT // 2) + 2 * pp
                            nc.vector.tensor_copy(
                                xT[:, kk:kk + 2, b * P:(b + 1) * P],
                                pst[:, :, :, 0])

                KH = KT // 4     # 8: first half of kp range (kp-major)
                KW = 4           # warm-up kp rows emitted between T phases
                for jh in range(2):
                    ps = {}
                    for j4 in range(4):
                        ps[(0, j4)] = psacc.tile(
                            [P, JN], f32, name=f"acc{jh}_0_{j4}",
                            tag=f"acc0_{j4}")
                    if jh == 0:
                        transposes(0)
                        for kp in range(KW):
                            mt = mask_tiles[(jh, kp)]
                            w = xT[:, 2 * kp:2 * kp + 2, 0:P]
                            for j4 in range(4):
                                nc.tensor.matmul(
                                    ps[(0, j4)][:], w,
                                    mt[:, :,
                                       j4 * JN:(j4 + 1) * JN].bitcast(f8),
                                    start=(kp == 0), stop=False,
                                    perf_mode=DR)
                        transposes(1)
                    for j4 in range(4):
                        ps[(1, j4)] = psacc.tile(
                            [P, JN], f32, name=f"acc{jh}_1_{j4}",
                            tag=f"acc1_{j4}")
                    if jh == 0:
                        for kp in range(KW):
                            mt = mask_tiles[(jh, kp)]
                            w = xT[:, 2 * kp:2 * kp + 2, P:2 * P]
                            for j4 in range(4):
                                nc.tensor.matmul(
                                    ps[(1, j4)][:], w,
                                    mt[:, :,
                                       j4 * JN:(j4 + 1) * JN].bitcast(f8),
                                    start=(kp == 0), stop=False,
                                    perf_mode=DR)
                    for kp in range(KW if jh == 0 else 0, KH):
                        mt = mask_tiles[(jh, kp)]
                        for b in range(NB):
                            w = xT[:, 2 * kp:2 * kp + 2, b * P:(b + 1) * P]
                            for j4 in range(4):
                                nc.tensor.matmul(
                                    ps[(b, j4)][:], w,
                                    mt[:, :,
                                       j4 * JN:(j4 + 1) * JN].bitcast(f8),
                                    start=(kp == 0), stop=False,
                                    perf_mode=DR)
                        if kp == 6:
                            # fold thresholds: psum -= eps*th
                            for b in range(NB):
                                for j4 in range(4):
                                    jj = jh * (J // 2) + j4 * JN
                                    nc.tensor.matmul(
                                        ps[(b, j4)][:], wstar[:],
                                        dig8[:, jj:jj + JN],
                                        start=False, stop=False,
                                        skip_group_check=True)
                    # second k-half group-major: groups retire staggered so
                    # the is_gt epilogue overlaps remaining matmuls
                    for b in range(NB):
                        for j4 in range(4):
                            jj = jh * (J // 2) + j4 * JN
                            w = None
                            for kp in range(KH, KT // 2):
                                nc.tensor.matmul(
                                    ps[(b, j4)][:],
                                    xT[:, 2 * kp:2 * kp + 2,
                                       b * P:(b + 1) * P],
                                    mask_tiles[(jh, kp)][
                                        :, :,
                                        j4 * JN:(j4 + 1) * JN].bitcast(f8),
                                    start=False, stop=(kp == KT // 2 - 1),
                                    perf_mode=DR)
                            nc.vector.tensor_scalar(
                                obs[b][:, jj:jj + JN], ps[(b, j4)][:],
                                rxe[:, b:b + 1], None, op0=ALU.is_gt)
                            nc.sync.dma_start(
                                o_d[b * P:(b + 1) * P, jj:jj + JN],
                                obs[b][:, jj:jj + JN])

    nc.compile()
    return nc


def _get_nc():
    if "nc" not in _cache:
        _cache["nc"] = _build()
    return _cache["nc"]


def _cst8():
    import ml_dtypes
    # eps-scaled digit weights: -eps*8^i per digit row (row 3 holds 8*d3,
    # so its weight is -eps*512/8 = -2^-3)
    w = np.array([-2.0 ** -9, -2.0 ** -6, -2.0 ** -3, -2.0 ** -3],
                 dtype=np.float32)
    return np.broadcast_to(w[:, None], (4, P)).astype(ml_dtypes.float8_e4m3)


def _ident():
    import ml_dtypes
    return np.eye(P, dtype=np.float32).astype(ml_dtypes.float8_e4m3)


def _csts():
    return np.array([[0, 7], [3, 7], [6, 7], [6, 56]], dtype=np.int32)


def run(x, masks, thresholds, trace=False):
    """Run the SPMD kernel on 8 cores. Returns (out_bool, BassKernelResults)."""
    from concourse.bass_utils import run_bass_kernel_spmd

    nc = _get_nc()
    m_u8 = np.ascontiguousarray(masks.view(np.uint8))
    th = np.ascontiguousarray(thresholds.reshape(1, J).astype(np.int32))
    in_maps = []
    for c in range(NCORES):
        in_maps.append({
            "x": np.ascontiguousarray(x[c * BL:(c + 1) * BL, :]),
            "masks": m_u8,
            "th": th,
            "cst8": _cst8(),
            "ident": _ident(),
            "csts": _csts(),
        })
    res = run_bass_kernel_spmd(nc, in_maps, core_ids=list(range(NCORES)),
                               trace=trace)
    out = np.concatenate([r["out"] for r in res.results], axis=0)
    return out.view(np.bool_), res


def kernel(x, masks, thresholds):
    x = np.asarray(x)
    masks = np.asarray(masks)
    thresholds = np.asarray(thresholds)
    out, _ = run(x, masks, thresholds, trace=False)
    return out
